# revision 28
# baseline (speedup 1.0000x reference)
"""DWHT (buggy in-place Walsh-Hadamard channel transform + channel shuffle) on 8 trn2 cores.

The whole nn.Module is a fixed linear map on the channel axis:
    y[b, :, h, w] = T @ x[b, :, h, w]
with T a (512, 256) matrix of small integers (|T| <= 13, exactly representable
in bf16).  Batch 64 is sharded 8-ways (data parallel, 8 samples/core); each
core runs a tiled PE matmul: for every sample, y_s (512,784) = T @ x_s (256,784).

Precision strategy (variant "bf16_split"): split x = hi + lo where hi is the
bf16 truncation of the fp32 payload (read for free via a strided bf16 view of
the upper 2 bytes of each fp32 word) and lo = bf16(x - hi) (one DVE op).  Both
products T@hi and T@lo are exact in fp32 PSUM accumulation, so the result
matches plain fp32 math to ~1e-6 while the PE streams at full bf16 rate.
"""

import os
import sys

import numpy as np

for _p in ("/opt/trn_rl_repo", "/root/.axon_site/_ro/trn_rl_repo"):
    if os.path.isdir(_p) and _p not in sys.path:
        sys.path.append(_p)

B, C_IN, C_OUT, HH, WW = 64, 256, 512, 28, 28
S = HH * WW  # 784
N_CORES = 8
BS = B // N_CORES  # 8 samples per core
N_PASSES, GROUPS = 8, 8

VARIANT = os.environ.get("DWHT_VARIANT", "fp16_lo")

# spatial split per PSUM bank (each chunk <= 512 fp32 = one bank)
N_CHUNKS = ((0, 392), (392, 392))


def _dwht_T() -> np.ndarray:
    """Build the (512, 256) transform matrix by running the reference
    butterfly (including its partial-update in-place semantics) on identity."""
    x = np.zeros((C_OUT, C_IN), np.float64)
    x[:C_IN] = np.eye(C_IN)
    half = C_OUT // 2
    for _ in range(N_PASSES):
        top = x[::2] + x[1::2]
        x = x.copy()
        x[:half] = top
        bottom = x[::2] - x[1::2]
        x[half:] = bottom
    # channel shuffle with groups=8
    x = x.reshape(GROUPS, C_OUT // GROUPS, C_IN).transpose(1, 0, 2).reshape(C_OUT, C_IN)
    return x


def _build(variant, reps=1):
    import concourse.mybir as mybir
    from concourse import bacc
    from concourse.tile import TileContext

    f32 = mybir.dt.float32
    bf16 = mybir.dt.bfloat16
    fp16 = mybir.dt.float16
    f32r = mybir.dt.float32r

    tt_dt = bf16 if variant in ("bf16_split", "fp16_lo") else f32

    nc = bacc.Bacc(None, target_bir_lowering=False)
    x = nc.dram_tensor("x", (BS, C_IN, S), f32, kind="ExternalInput")
    tt = nc.dram_tensor("tt", (C_IN, C_OUT), tt_dt, kind="ExternalInput")
    y = nc.dram_tensor("y", (BS, C_OUT, S), f32, kind="ExternalOutput")

    with TileContext(nc) as tc:
        with (
            tc.tile_pool(name="w", bufs=1) as wp,
            tc.tile_pool(name="io", bufs=3) as io,
            tc.tile_pool(name="ps", bufs=8, space="PSUM") as pp,
        ):
            tts = []
            ttr = []
            tth = []
            for k in range(2):
                t = wp.tile([128, C_OUT], tt_dt, tag=f"tt{k}")
                nc.sync.dma_start(out=t[:], in_=tt[k * 128 : (k + 1) * 128, :])
                tts.append(t)
                if variant == "f32r":
                    r = wp.tile([128, C_OUT], f32r, tag=f"ttr{k}")
                    nc.vector.tensor_copy(r[:], t[:])
                    ttr.append(r)
                if variant == "fp16_lo":
                    # T is small integers: exact in fp16 as well
                    h = wp.tile([128, C_OUT], fp16, tag=f"tth{k}")
                    nc.vector.tensor_copy(h[:], t[:])
                    tth.append(h)

            for s in [s for _ in range(reps) for s in range(BS)]:
                xsk = []
                for k in range(2):
                    xs = io.tile([128, S], f32, tag="xs", bufs=8)
                    nc.scalar.dma_start(out=xs[:], in_=x[s, k * 128 : (k + 1) * 128, :])
                    xsk.append(xs)

                # passes: list of (rhs_ap, weights_tile) accumulated into PSUM
                if variant in ("bf16_split", "fp16_lo"):
                    lo_dt = bf16 if variant == "bf16_split" else fp16
                    lo_tt = tts if variant == "bf16_split" else tth
                    # hi: upper 2 bytes of each little-endian fp32 word
                    xhk = [
                        xs.bitcast(bf16).rearrange("p (f two) -> p f two", two=2)[
                            :, :, 1
                        ]
                        for xs in xsk
                    ]
                    xlk = []
                    for k in range(2):
                        xl = io.tile([128, S], lo_dt, tag="xl", bufs=8)
                        nc.vector.tensor_sub(xl[:], xsk[k][:], xhk[k])
                        xlk.append(xl)
                    # xl first: the group-opening matmul's psum-slot-release
                    # wait and its rhs-ready wait are then the same DVE sem
                    # (the MM ISA slot can encode only one sync wait).
                    passes = [
                        (xlk[0][:], lo_tt[0]),
                        (xhk[0], tts[0]),
                        (xlk[1][:], lo_tt[1]),
                        (xhk[1], tts[1]),
                    ]
                elif variant == "f32r":
                    xrk = []
                    for k in range(2):
                        xr = io.tile([128, S], f32r, tag="xr", bufs=8)
                        nc.vector.tensor_copy(xr[:], xsk[k][:])
                        xrk.append(xr)
                    passes = [(xrk[0][:], ttr[0]), (xrk[1][:], ttr[1])]
                elif variant == "f32":
                    passes = [(xsk[0][:], tts[0]), (xsk[1][:], tts[1])]
                else:
                    raise ValueError(variant)

                combine = os.environ.get("DWHT_OUT_COMBINE", "1") == "1"
                for m in range(C_OUT // 128):
                    msl = slice(m * 128, (m + 1) * 128)
                    ysm = None
                    if combine:
                        ysm = io.tile([128, S], f32, tag="ysm", bufs=4, name="ysm")
                    for ni, (n0, nsz) in enumerate(N_CHUNKS):
                        nsl = slice(n0, n0 + nsz)
                        ps = pp.tile([128, nsz], f32, tag="ps")
                        for i, (src, w) in enumerate(passes):
                            nc.tensor.matmul(
                                ps[:],
                                w[:, msl],
                                src[:, nsl],
                                start=(i == 0),
                                stop=(i == len(passes) - 1),
                            )
                        if combine:
                            ys = ysm[:, nsl]
                        else:
                            ys_t = io.tile([128, nsz], f32, tag="ys", bufs=6, name="ys")
                            ys = ys_t[:]
                        # balance PSUM->SBUF copies across DVE and ACT
                        if (m * len(N_CHUNKS) + ni) % 2 == 0:
                            nc.vector.tensor_copy(ys, ps[:])
                        else:
                            nc.scalar.copy(ys, ps[:])
                        if not combine:
                            nc.sync.dma_start(out=y[s, msl, nsl], in_=ys)
                    if combine:
                        nc.sync.dma_start(out=y[s, msl, :], in_=ysm[:])

    nc.compile()
    return nc


_cache = {}


def _get_nc(variant, reps=1):
    key = (variant, reps)
    if key not in _cache:
        _cache[key] = _build(variant, reps)
    return _cache[key]


def _in_maps(x_np, variant):
    import ml_dtypes

    T = _dwht_T()
    ttT = np.ascontiguousarray(T.T)  # (256, 512), lhsT layout
    if variant in ("bf16_split", "fp16_lo"):
        tt_np = ttT.astype(ml_dtypes.bfloat16)
    else:
        tt_np = ttT.astype(np.float32)
    return [
        {"x": x_np[i * BS : (i + 1) * BS], "tt": tt_np} for i in range(N_CORES)
    ]


def _run(x_np, variant=None, trace=False, reps=1):
    from concourse.bass_utils import run_bass_kernel_spmd

    variant = variant or VARIANT
    nc = _get_nc(variant, reps)
    res = run_bass_kernel_spmd(
        nc, _in_maps(x_np, variant), list(range(N_CORES)), trace=trace
    )
    y = np.stack([r["y"] for r in res.results]).reshape(B, C_OUT, HH, WW)
    return y, res


def kernel(x: np.ndarray) -> np.ndarray:
    x_np = np.ascontiguousarray(np.asarray(x), dtype=np.float32).reshape(B, C_IN, S)
    y, _ = _run(x_np)
    return y


# revision 29
# speedup vs baseline: 1.0166x; 1.0166x over previous
"""DWHT (buggy in-place Walsh-Hadamard channel transform + channel shuffle) on 8 trn2 cores.

The whole nn.Module is a fixed linear map on the channel axis:
    y[b, :, h, w] = T @ x[b, :, h, w]
with T a (512, 256) matrix of small integers (|T| <= 13, exactly representable
in bf16).  Batch 64 is sharded 8-ways (data parallel, 8 samples/core); each
core runs a tiled PE matmul: for every sample, y_s (512,784) = T @ x_s (256,784).

Precision strategy (variant "bf16_split"): split x = hi + lo where hi is the
bf16 truncation of the fp32 payload (read for free via a strided bf16 view of
the upper 2 bytes of each fp32 word) and lo = bf16(x - hi) (one DVE op).  Both
products T@hi and T@lo are exact in fp32 PSUM accumulation, so the result
matches plain fp32 math to ~1e-6 while the PE streams at full bf16 rate.
"""

import os
import sys

import numpy as np

for _p in ("/opt/trn_rl_repo", "/root/.axon_site/_ro/trn_rl_repo"):
    if os.path.isdir(_p) and _p not in sys.path:
        sys.path.append(_p)

B, C_IN, C_OUT, HH, WW = 64, 256, 512, 28, 28
S = HH * WW  # 784
N_CORES = 8
BS = B // N_CORES  # 8 samples per core
N_PASSES, GROUPS = 8, 8

VARIANT = os.environ.get("DWHT_VARIANT", "fp16_lo")

# spatial split per PSUM bank (each chunk <= 512 fp32 = one bank)
N_CHUNKS = ((0, 392), (392, 392))


def _dwht_T() -> np.ndarray:
    """Build the (512, 256) transform matrix by running the reference
    butterfly (including its partial-update in-place semantics) on identity."""
    x = np.zeros((C_OUT, C_IN), np.float64)
    x[:C_IN] = np.eye(C_IN)
    half = C_OUT // 2
    for _ in range(N_PASSES):
        top = x[::2] + x[1::2]
        x = x.copy()
        x[:half] = top
        bottom = x[::2] - x[1::2]
        x[half:] = bottom
    # channel shuffle with groups=8
    x = x.reshape(GROUPS, C_OUT // GROUPS, C_IN).transpose(1, 0, 2).reshape(C_OUT, C_IN)
    return x


def _build(variant, reps=1):
    import concourse.mybir as mybir
    from concourse import bacc
    from concourse.tile import TileContext

    f32 = mybir.dt.float32
    bf16 = mybir.dt.bfloat16
    fp16 = mybir.dt.float16
    f32r = mybir.dt.float32r

    tt_dt = bf16 if variant in ("bf16_split", "fp16_lo") else f32

    nc = bacc.Bacc(None, target_bir_lowering=False)
    x = nc.dram_tensor("x", (BS, C_IN, S), f32, kind="ExternalInput")
    tt = nc.dram_tensor("tt", (C_IN, C_OUT), tt_dt, kind="ExternalInput")
    y = nc.dram_tensor("y", (BS, C_OUT, S), f32, kind="ExternalOutput")

    with TileContext(nc) as tc:
        with (
            tc.tile_pool(name="w", bufs=1) as wp,
            tc.tile_pool(name="io", bufs=3) as io,
            tc.tile_pool(name="ps", bufs=8, space="PSUM") as pp,
        ):
            tts = []
            ttr = []
            tth = []
            for k in range(2):
                t = wp.tile([128, C_OUT], tt_dt, tag=f"tt{k}")
                nc.sync.dma_start(out=t[:], in_=tt[k * 128 : (k + 1) * 128, :])
                tts.append(t)
                if variant == "f32r":
                    r = wp.tile([128, C_OUT], f32r, tag=f"ttr{k}")
                    nc.vector.tensor_copy(r[:], t[:])
                    ttr.append(r)
                if variant == "fp16_lo":
                    # T is small integers: exact in fp16 as well
                    h = wp.tile([128, C_OUT], fp16, tag=f"tth{k}")
                    nc.vector.tensor_copy(h[:], t[:])
                    tth.append(h)

            for s in [s for _ in range(reps) for s in range(BS)]:
                xsk = []
                for k in range(2):
                    xs = io.tile([128, S], f32, tag="xs", bufs=8)
                    nc.scalar.dma_start(out=xs[:], in_=x[s, k * 128 : (k + 1) * 128, :])
                    xsk.append(xs)

                # passes: list of (rhs_ap, weights_tile) accumulated into PSUM
                if variant in ("bf16_split", "fp16_lo"):
                    lo_dt = bf16 if variant == "bf16_split" else fp16
                    lo_tt = tts if variant == "bf16_split" else tth
                    # hi: upper 2 bytes of each little-endian fp32 word
                    xhk = [
                        xs.bitcast(bf16).rearrange("p (f two) -> p f two", two=2)[
                            :, :, 1
                        ]
                        for xs in xsk
                    ]
                    xlk = []
                    for k in range(2):
                        xl = io.tile([128, S], lo_dt, tag="xl", bufs=8)
                        nc.vector.tensor_sub(xl[:], xsk[k][:], xhk[k])
                        xlk.append(xl)
                    # xl first: the group-opening matmul's psum-slot-release
                    # wait and its rhs-ready wait are then the same DVE sem
                    # (the MM ISA slot can encode only one sync wait).
                    passes = [
                        (xlk[0][:], lo_tt[0]),
                        (xhk[0], tts[0]),
                        (xlk[1][:], lo_tt[1]),
                        (xhk[1], tts[1]),
                    ]
                elif variant == "f32r":
                    xrk = []
                    for k in range(2):
                        xr = io.tile([128, S], f32r, tag="xr", bufs=8)
                        nc.vector.tensor_copy(xr[:], xsk[k][:])
                        xrk.append(xr)
                    passes = [(xrk[0][:], ttr[0]), (xrk[1][:], ttr[1])]
                elif variant == "f32":
                    passes = [(xsk[0][:], tts[0]), (xsk[1][:], tts[1])]
                else:
                    raise ValueError(variant)

                for m in range(C_OUT // 128):
                    msl = slice(m * 128, (m + 1) * 128)
                    ysm = io.tile([128, S], f32, tag="ysm", bufs=4, name="ysm")
                    for ni, (n0, nsz) in enumerate(N_CHUNKS):
                        nsl = slice(n0, n0 + nsz)
                        ps = pp.tile([128, nsz], f32, tag="ps")
                        for i, (src, w) in enumerate(passes):
                            nc.tensor.matmul(
                                ps[:],
                                w[:, msl],
                                src[:, nsl],
                                start=(i == 0),
                                stop=(i == len(passes) - 1),
                            )
                        # balance PSUM->SBUF copies across DVE and ACT
                        if (m * len(N_CHUNKS) + ni) % 2 == 0:
                            nc.vector.tensor_copy(ysm[:, nsl], ps[:])
                        else:
                            nc.scalar.copy(ysm[:, nsl], ps[:])
                    nc.sync.dma_start(out=y[s, msl, :], in_=ysm[:])

    nc.compile()
    return nc


_cache = {}


def _get_nc(variant, reps=1):
    key = (variant, reps)
    if key not in _cache:
        _cache[key] = _build(variant, reps)
    return _cache[key]


def _in_maps(x_np, variant):
    import ml_dtypes

    T = _dwht_T()
    ttT = np.ascontiguousarray(T.T)  # (256, 512), lhsT layout
    if variant in ("bf16_split", "fp16_lo"):
        tt_np = ttT.astype(ml_dtypes.bfloat16)
    else:
        tt_np = ttT.astype(np.float32)
    return [
        {"x": x_np[i * BS : (i + 1) * BS], "tt": tt_np} for i in range(N_CORES)
    ]


def _run(x_np, variant=None, trace=False, reps=1):
    from concourse.bass_utils import run_bass_kernel_spmd

    variant = variant or VARIANT
    nc = _get_nc(variant, reps)
    res = run_bass_kernel_spmd(
        nc, _in_maps(x_np, variant), list(range(N_CORES)), trace=trace
    )
    y = np.stack([r["y"] for r in res.results]).reshape(B, C_OUT, HH, WW)
    return y, res


def kernel(x: np.ndarray) -> np.ndarray:
    x_np = np.ascontiguousarray(np.asarray(x), dtype=np.float32).reshape(B, C_IN, S)
    y, _ = _run(x_np)
    return y
